# revision 1
# baseline (speedup 1.0000x reference)
"""Causal single-head attention (B=4, T=4096, D_in=1024, D_out=64) on 8 trn2 cores.

Sharding: 2 cores per batch. Within a pair, core h in {0,1} owns the k/v
positions in 256-wide blocks of parity h (even/odd), and computes partial
unnormalized attention for ALL 4096 queries over its k half, plus the
softmax row-sums (via a ones-column appended to V). The host sums the two
partials and normalizes. Causality lands symmetrically on both parities, so
one SPMD program (identical instruction stream) serves all 8 cores; per-core
behavior differs only through data:

  - xT (x[b] transposed to [D,T]) with each 512-column tile's two 256-blocks
    swapped for h=1, so "even permuted block" = own-parity block on every core
  - iql (local q index per slot column; h=1 columns are block-swapped, so
    iql[c] = c ^ 256 there) and kgl (local key index per diagonal k-tile row)
    driving the causal mask compare

The whole pipeline runs in fp16 with fp32 PSUM accumulation (~7e-4 rel
err; the 1/8 softmax scale absorbs fp16 score rounding). Projection stripes
(1024 t-columns) interleave with attention q-slots so PE stays warm and DMA
overlaps compute; DMA triggers are spread across the sync/scalar/gpsimd
queues so small latency-critical transfers don't queue behind the 2MB
stripe streams (HWDGE rings are FIFO per issuing engine). A short burst of
dependency-free fp16 junk matmuls at t=0 holds the PE HAM clock-gate open
through the first-stripe DMA window.

Attention per q-slot qt (512 queries, 2*qt+2 k-tiles of 128): scores use
c=64 contraction, two k-tiles packed in the PE row halves (tile_position
(0,0)/(64,0)), K^T and Q^T duplicated into partitions 64..127. The diagonal
(masked) pair runs FIRST and its second k-tile is trimmed to the causally
needed 384 q-columns (scores, exp, mask, attnV all narrower). Masks compare
slot-local q index (iql, fp16) against per-core local key index (kgl, fp16)
so the mask stt runs at the DVE 16-bit rate. attnV is m=65 (V plus a ones
column for the softmax denominator), lagging one pair behind scores so PE
never waits on ACT.
"""

import sys
import types

import numpy as np

B, T, D, E = 4, 4096, 1024, 64
NCORES = 8
P = 128
HB = 256  # parity half-block width
NQT = 8  # q-slots of 512
DC = D // P  # 8 d-chunks

_cache = {}


def _sl(start, size):
    return slice(start, start + size)


def _build_program():
    import concourse.mybir as mybir
    import concourse.tile as tile
    from concourse import bacc

    f32 = mybir.dt.float32
    fp16 = mybir.dt.float16
    Exp = mybir.ActivationFunctionType.Exp
    Alu = mybir.AluOpType

    nc = bacc.Bacc("TRN2", target_bir_lowering=False, debug=False, num_devices=NCORES)

    xT = nc.dram_tensor("xT", [16, P, DC, HB], fp16, kind="ExternalInput")
    wkv = nc.dram_tensor("wkv", [P, DC, 2 * E], fp16, kind="ExternalInput")
    wq2 = nc.dram_tensor("wq2", [P, DC, P], fp16, kind="ExternalInput")
    kgl = nc.dram_tensor("kgl", [P, 2], fp16, kind="ExternalInput")
    iql = nc.dram_tensor("iql", [P, 512], fp16, kind="ExternalInput")
    ident = nc.dram_tensor("ident", [P, 64], fp16, kind="ExternalInput")
    ones = nc.dram_tensor("ones", [P, 16], fp16, kind="ExternalInput")
    out = nc.dram_tensor("out", [E + 1, T], f32, kind="ExternalOutput")

    with tile.TileContext(nc) as tc:
        with (
            tc.tile_pool(name="const", bufs=1) as cpool,
            tc.tile_pool(name="persist", bufs=1) as ppool,
            tc.tile_pool(name="xt", bufs=2) as xtpool,
            tc.tile_pool(name="kvps", bufs=2, space="PSUM") as kvps,
            tc.tile_pool(name="sps", bufs=2, space="PSUM") as sps,
            tc.tile_pool(name="ops", bufs=2, space="PSUM") as ops,
            tc.tile_pool(name="exp", bufs=4) as exppool,
        ):
            kT_sb = ppool.tile([P, T // 2], fp16, name="kT")  # rows 64+: dup
            qT_sb = ppool.tile([P, T], fp16, name="qT")  # rows 64+: dup
            vT_tmp = ppool.tile([P, T // 2], fp16, name="vTt")  # rows 64+ used
            V_sb = ppool.tile([P, 16, E + 1], fp16, name="V")
            out_sb = ppool.tile([E + 1, T], f32, name="outsb")

            # PE warm-up: junk matmuls on a memset tile issue immediately
            # and hold the HAM clock-gate open through the first-stripe DMA
            # window (fp16 n=512 ~ 630ns each cold).
            junk_in = ppool.tile([P, 512], fp16, name="junkin")
            nc.vector.memset(junk_in[:], 0.0)
            warm = ops.tile([E + 1, 512], f32, tag="po")
            for _ in range(10):
                nc.tensor.matmul(
                    warm[0:64, :],
                    junk_in[:, 0:E],
                    junk_in[:],
                    start=True,
                    stop=True,
                )

            # weights first (first consumers) on the scalar queue, then
            # stripe 0 quarters on sync, then the rest
            wkv_sb = cpool.tile([P, DC, 2 * E], fp16)
            nc.scalar.dma_start(wkv_sb[:], wkv.ap())
            wq2_sb = cpool.tile([P, DC, P], fp16)
            nc.scalar.dma_start(wq2_sb[:], wq2.ap())

            stripes = [None] * 4  # per-stripe xt tiles

            def issue_stripe_dma(t2, split=False, eng=None):
                # xT is pre-tiled host-side as [16 quarters, P, DC, 256] so
                # every transfer reads 4KB contiguous runs per partition
                if split:
                    quarters = []
                    for hh in range(4):
                        xt_h = xtpool.tile([P, DC, HB], fp16, tag=f"xt0{hh}")
                        nc.sync.dma_start(xt_h[:], xT.ap()[4 * t2 + hh])
                        quarters.append(xt_h)
                    stripes[t2] = quarters
                else:
                    xt_t = xtpool.tile([P, 4, DC, HB], fp16)
                    (eng or nc.sync).dma_start(
                        xt_t[:],
                        xT.ap()[_sl(4 * t2, 4)].rearrange("q p c o -> p q c o"),
                    )
                    stripes[t2] = xt_t

            issue_stripe_dma(0, split=True)
            issue_stripe_dma(1)

            # small constants on the scalar queue (gpsimd's library-load
            # preamble would delay anything issued there early)
            kgl_sb = cpool.tile([P, 2], fp16)
            nc.scalar.dma_start(kgl_sb[:], kgl.ap())
            iql_sb = cpool.tile([P, 512], fp16)
            nc.scalar.dma_start(iql_sb[:], iql.ap())
            ident_sb = cpool.tile([P, 64], fp16)
            nc.scalar.dma_start(ident_sb[:], ident.ap())
            nc.gpsimd.dma_start(V_sb[:, :, E], ones.ap())  # ones column

            def issue_stripe_proj(t2):
                xts = stripes[t2]
                if t2 == 0:
                    # first stripe lands as four independent quarter tiles:
                    # kv block m comes entirely from quarter 2m, so slot 0
                    # can start as soon as the first 0.5MB arrives
                    for half in range(2):
                        xth = xts[2 * half]
                        kvh = kvps.tile([P, HB], f32, tag="proj")
                        for dc in range(DC):
                            nc.tensor.matmul(
                                kvh[:],
                                wkv_sb[:, dc, :],
                                xth[:, dc, :],
                                start=(dc == 0),
                                stop=(dc == DC - 1),
                            )
                        m = half
                        nc.vector.tensor_copy(
                            kT_sb[0:E, _sl(HB * m, HB)], kvh[0:E, :]
                        )
                        nc.vector.tensor_copy(
                            vT_tmp[E:P, _sl(HB * m, HB)], kvh[E:P, :]
                        )
                        # early per-block K^T dup so slot masks aren't gated
                        # on the whole stripe
                        nc.scalar.dma_start(
                            kT_sb[E:P, _sl(HB * m, HB)],
                            kT_sb[0:E, _sl(HB * m, HB)],
                        )
                else:
                    # K^T|V^T: one MM per d-chunk, rhs covers both parity
                    # blocks of the stripe via a strided (2,256) pattern
                    kv = kvps.tile([P, 512], f32, tag="proj")
                    xts_v = xts[:, :, :, :].rearrange(
                        "p (h par) c o -> p h par c o", h=2
                    )
                    for dc in range(DC):
                        nc.tensor.matmul(
                            kv[:],
                            wkv_sb[:, dc, :],
                            xts_v[:, :, 0, dc, :],
                            start=(dc == 0),
                            stop=(dc == DC - 1),
                        )
                    m = 2 * t2
                    nc.vector.tensor_copy(kT_sb[0:E, _sl(HB * m, 512)], kv[0:E, :])
                    nc.vector.tensor_copy(vT_tmp[E:P, _sl(HB * m, 512)], kv[E:P, :])
                for half in range(2):
                    # Q^T over the full 512-tile (doubled weights -> rows
                    # 64..127 carry a duplicate for row-packed scores)
                    q = kvps.tile([P, 512], f32, tag="proj")
                    if t2 == 0:
                        for qq in range(2):
                            for dc in range(DC):
                                nc.tensor.matmul(
                                    q[:, _sl(HB * qq, HB)],
                                    wq2_sb[:, dc, :],
                                    xts[2 * half + qq][:, dc, :],
                                    start=(dc == 0),
                                    stop=(dc == DC - 1),
                                )
                    else:
                        for dc in range(DC):
                            nc.tensor.matmul(
                                q[:],
                                wq2_sb[:, dc, :],
                                xts[:, _sl(2 * half, 2), dc, :],
                                start=(dc == 0),
                                stop=(dc == DC - 1),
                            )
                    qt_i = 2 * t2 + half
                    nc.vector.tensor_copy(qT_sb[:, _sl(512 * qt_i, 512)], q[:])
                if t2 != 0:
                    # duplicate the stripe's K^T into partitions 64..127
                    nc.scalar.dma_start(
                        kT_sb[E:P, _sl(512 * t2, 512)],
                        kT_sb[0:E, _sl(512 * t2, 512)],
                    )
                # V^T -> V via PE transpose (4 x 128-col pieces); kvps pool
                # so scores double-buffering in sps is never blocked
                for j in range(4 * t2, 4 * t2 + 4):
                    vt = kvps.tile([P, E], fp16, tag="proj")
                    nc.tensor.transpose(
                        vt[:], vT_tmp[E:P, _sl(P * j, P)], ident_sb[E:P, :]
                    )
                    nc.vector.tensor_copy(V_sb[:, j, 0:E], vt[:])

            pendings = []  # (qt, oi, j0, g, nkb, exp_tile, po, w2)

            def issue_attnv(pend):
                qt, oi, j0, g, nkb, ex, po_t, w2 = pend
                n_groups = nkb // 2
                nc.tensor.matmul(
                    po_t[:],
                    V_sb[:, j0, :],
                    ex[:, 0:512],
                    start=(oi == 0),
                    stop=False,
                )
                nc.tensor.matmul(
                    po_t[:, 512 - w2 : 512],
                    V_sb[:, j0 + 1, :],
                    ex[:, _sl(512, w2)],
                    start=False,
                    stop=(oi == n_groups - 1),
                )
                if oi == n_groups - 1:
                    nc.vector.tensor_copy(out_sb[:, _sl(512 * qt, 512)], po_t[:])
                    nc.sync.dma_start(
                        out.ap()[:, _sl(512 * qt, 512)],
                        out_sb[:, _sl(512 * qt, 512)],
                    )

            def issue_slot(qt):
                nkb = 2 * qt + 2
                po = ops.tile([E + 1, 512], f32)
                # diagonal (masked) pair first: its mask latency hides
                # behind the remaining unmasked pairs. Its second k-tile
                # only covers q-columns 128.. (h=0 parity; h=1 keys sit
                # higher so the extra columns are masked anyway): trim
                # scores/exp/attnV to 384 columns.
                order = [nkb - 2] + list(range(0, nkb - 2, 2))
                for oi, j0 in enumerate(order):
                    w2 = 384 if oi == 0 else 512
                    ps = sps.tile([P, 1024], f32, tag="ps")
                    # two c=64 score matmuls in the PE's row halves
                    nc.tensor.matmul(
                        ps[:, 0:512],
                        kT_sb[0:E, _sl(P * j0, P)],
                        qT_sb[0:E, _sl(512 * qt, 512)],
                        start=True,
                        stop=True,
                        tile_position=(0, 0),
                    )
                    nc.tensor.matmul(
                        ps[:, _sl(512, w2)],
                        kT_sb[E:P, _sl(P * (j0 + 1), P)],
                        qT_sb[E:P, _sl(512 * qt + (512 - w2), w2)],
                        start=True,
                        stop=True,
                        tile_position=(64, 0),
                    )
                    ex = exppool.tile([P, 1024], fp16)
                    nc.scalar.activation(
                        ex[:, : 512 + w2], ps[:, : 512 + w2], Exp, scale=0.125
                    )
                    if oi == 0:
                        # the diagonal pair: the only k-tiles needing a mask
                        nc.vector.scalar_tensor_tensor(
                            out=ex[:, 0:512],
                            in0=iql_sb[:, 0:512],
                            scalar=kgl_sb[:, 0:1],
                            in1=ex[:, 0:512],
                            op0=Alu.is_ge,
                            op1=Alu.mult,
                        )
                        nc.vector.scalar_tensor_tensor(
                            out=ex[:, _sl(512, w2)],
                            in0=iql_sb[:, _sl(512 - w2, w2)],
                            scalar=kgl_sb[:, 1:2],
                            in1=ex[:, _sl(512, w2)],
                            op0=Alu.is_ge,
                            op1=Alu.mult,
                        )
                    pendings.append((qt, oi, j0, 2, nkb, ex, po, w2))
                    if len(pendings) > 1:
                        issue_attnv(pendings.pop(0))

            # --- schedule: stripes interleaved with attention slots
            issue_stripe_proj(0)
            for seg in range(4):
                if seg < 2:
                    issue_stripe_dma(seg + 2, eng=nc.gpsimd if seg == 1 else None)
                issue_slot(2 * seg)
                if seg < 3:
                    issue_stripe_proj(seg + 1)
                issue_slot(2 * seg + 1)
            for pend in pendings:
                issue_attnv(pend)

    nc.compile()
    return nc


def _host_inputs():
    """Core-independent pieces + per-parity local mask tables (iql, kgl)."""
    ident = np.zeros((P, 64), dtype=np.float32)
    for p in range(P):
        ident[p, p % 64] = 1.0
    # local q index per slot column, bcast over rows. On h=1 cores the qT
    # columns are 256-block-swapped within each 512 window, so the local q
    # of column c is c ^ 256.
    iqls = []
    for h in range(2):
        col = np.arange(512, dtype=np.int64)
        if h == 1:
            col = col ^ 256
        iqls.append(
            np.broadcast_to(col.astype(np.float32), (P, 512)).copy()
        )
    # local key index of the two diagonal k-tiles, per parity h
    kgls = []
    ii = np.arange(P, dtype=np.float32)
    for h in range(2):
        kgl = np.zeros((P, 2), dtype=np.float32)
        kgl[:, 0] = HB * h + ii
        kgl[:, 1] = HB * h + P + ii
        kgls.append(kgl)
    return ident, iqls, kgls


def _ensure_axon_hooks_stub():
    """bass_utils imports antenv.axon_hooks when BASS_TRACE is set; that
    module is absent in this image, so provide a no-op registry."""
    try:
        import antenv.axon_hooks  # noqa: F401
    except ImportError:
        m = types.ModuleType("antenv.axon_hooks")
        m._h = [None]
        m.set_axon_ntff_profile_hook = lambda h: m._h.__setitem__(0, h)
        m.get_axon_ntff_profile_hook = lambda: m._h[0]
        sys.modules["antenv.axon_hooks"] = m


def kernel(x, Wq, Wk, Wv):
    _ensure_axon_hooks_stub()
    from concourse.bass_utils import run_bass_kernel_spmd

    if "nc" not in _cache:
        _cache["nc"] = _build_program()
    nc = _cache["nc"]

    x = np.asarray(x, dtype=np.float32)
    Wq = np.asarray(Wq, dtype=np.float32)
    Wk = np.asarray(Wk, dtype=np.float32)
    Wv = np.asarray(Wv, dtype=np.float32)

    wkv = np.ascontiguousarray(
        np.concatenate([Wk, Wv], axis=1)
        .reshape(DC, P, 2 * E)
        .transpose(1, 0, 2)
        .astype(np.float16)
    )
    wq2 = np.ascontiguousarray(
        np.concatenate([Wq, Wq], axis=1)
        .reshape(DC, P, P)
        .transpose(1, 0, 2)
        .astype(np.float16)
    )
    ident, iqls, kgls = _host_inputs()
    ident = ident.astype(np.float16)
    ones = np.ones((P, 16), dtype=np.float16)

    xT_all = x.transpose(0, 2, 1).astype(np.float16)  # [B, D, T]
    in_maps = []
    for c in range(NCORES):
        b, h = c // 2, c % 2
        xT = xT_all[b]
        if h == 1:  # swap 256-pairs so own-parity block is at even positions
            xT = xT.reshape(D, 8, 2, HB)[:, :, ::-1, :].reshape(D, T)
        # pre-tile to [16 quarters, P, DC, 256] for contiguous DMA runs
        xTq = xT.reshape(DC, P, 16, HB).transpose(2, 1, 0, 3)
        in_maps.append(
            {
                "xT": np.ascontiguousarray(xTq),
                "wkv": wkv,
                "wq2": wq2,
                "kgl": kgls[h].astype(np.float16),
                "iql": iqls[h].astype(np.float16),
                "ident": ident,
                "ones": ones,
            }
        )

    res = run_bass_kernel_spmd(nc, in_maps, list(range(NCORES)))
    _cache["last_res"] = res

    outp = np.empty((B, T, E), dtype=np.float32)
    for b in range(B):
        U = np.zeros((E + 1, T), dtype=np.float64)
        for h in range(2):
            u = res.results[2 * b + h]["out"].astype(np.float64)
            if h == 1:
                u = u.reshape(E + 1, 8, 2, HB)[:, :, ::-1, :].reshape(E + 1, T)
            U += u
        outp[b] = (U[:E] / U[E : E + 1]).T.astype(np.float32)
    return outp



# revision 5
# speedup vs baseline: 1.1390x; 1.1390x over previous
"""Causal single-head attention (B=4, T=4096, D_in=1024, D_out=64) on 8 trn2 cores.

Sharding: 2 cores per batch. Within a pair, core h in {0,1} owns the k/v
positions in 256-wide blocks of parity h (even/odd), and computes partial
unnormalized attention for ALL 4096 queries over its k half, plus the
softmax row-sums (via a ones-column appended to V). The host sums the two
partials and normalizes. Causality lands symmetrically on both parities, so
one SPMD program serves all 8 cores; per-core behavior differs only through
data (the parity block swap baked into the host-side xT layout, and maskD,
the precomputed diagonal-pair causal mask).

Key scheduling choices (v2):
  - x stripes ship as per-stripe DRAM tensors laid out [P, 4, DC, HB] so
    every partition reads one 16KB contiguous run (large DMA descriptors;
    ~2x effective HBM bandwidth vs 4KB quartered reads). Stripe 0 is split
    into two 8KB-run halves (quarters 01 / 23) so compute starts earlier.
  - DMA ring discipline: the sync (SP-HWDGE) ring carries weights + all x
    stripes + the V xbar-transposes + the final out store, in need order.
    The scalar (ACT-HWDGE) ring carries only wq2 so ACTIVATE (exp) is never
    queued behind a DMA trigger. gpsimd (SWDGE) carries the small constants,
    the kT row-duplications (SBUF->SBUF) and the early out stores.
  - V is produced via DMA xbar-transpose ([64, 512] fp16 -> [128, 4, 64])
    instead of PE transposes: frees ~4.5us of PE time.
  - The diagonal-pair causal mask is one precomputed [P, 896] fp16 0/1
    tensor applied with a single DVE multiply per slot.
  - Junk fp16 matmuls at t=0 hold the PE HAM clock-gate open until the
    first stripe lands; projections and attention pairs then keep PE busy
    continuously so HAM never re-throttles.

Attention per q-slot qt (512 queries, 2*qt+2 k-tiles of 128): scores use
c=64 contraction, two k-tiles packed in the PE row halves (tile_position
(0,0)/(64,0)), K^T and Q^T duplicated into partitions 64..127 (K^T by DMA
dup, Q^T free via doubled Wq columns). The diagonal (masked) pair runs
FIRST with its second k-tile trimmed to 384 q-columns. attnV is m=65
(V plus a ones column for the softmax denominator), lagging one pair
behind scores so PE never waits on ACT.
"""

import sys
import types

import numpy as np

B, T, D, E = 4, 4096, 1024, 64
NCORES = 8
P = 128
HB = 256  # parity half-block width
NQT = 8  # q-slots of 512
DC = D // P  # 8 d-chunks
NJUNK = 7

_cache = {}


def _sl(start, size):
    return slice(start, start + size)


def _build_program():
    import concourse.mybir as mybir
    import concourse.tile as tile
    from concourse import bacc

    f32 = mybir.dt.float32
    fp16 = mybir.dt.float16
    Exp = mybir.ActivationFunctionType.Exp
    Alu = mybir.AluOpType

    nc = bacc.Bacc("TRN2", target_bir_lowering=False, debug=False, num_devices=NCORES)

    xs0a = nc.dram_tensor("xs0a", [P, 2, DC, HB], fp16, kind="ExternalInput")
    xs0b = nc.dram_tensor("xs0b", [P, 2, DC, HB], fp16, kind="ExternalInput")
    xs1 = nc.dram_tensor("xs1", [P, 4, DC, HB], fp16, kind="ExternalInput")
    xs2 = nc.dram_tensor("xs2", [P, 4, DC, HB], fp16, kind="ExternalInput")
    xs3 = nc.dram_tensor("xs3", [P, 4, DC, HB], fp16, kind="ExternalInput")
    wkv = nc.dram_tensor("wkv", [P, DC, 2 * E], fp16, kind="ExternalInput")
    wq2 = nc.dram_tensor("wq2", [P, DC, P], fp16, kind="ExternalInput")
    maskD = nc.dram_tensor("maskD", [P, 896], fp16, kind="ExternalInput")
    ones = nc.dram_tensor("ones", [P, 16], fp16, kind="ExternalInput")
    out = nc.dram_tensor("out", [E + 1, T], f32, kind="ExternalOutput")

    with tile.TileContext(nc) as tc:
        with (
            tc.tile_pool(name="const", bufs=1) as cpool,
            tc.tile_pool(name="persist", bufs=1) as ppool,
            tc.tile_pool(name="xt0", bufs=2) as xt0pool,
            tc.tile_pool(name="xt", bufs=3) as xtpool,
            tc.tile_pool(name="vs", bufs=2) as vspool,
            tc.tile_pool(name="kvps", bufs=2, space="PSUM") as kvps,
            tc.tile_pool(name="sps", bufs=2, space="PSUM") as sps,
            tc.tile_pool(name="ops", bufs=2, space="PSUM") as ops,
            tc.tile_pool(name="exp", bufs=4) as exppool,
        ):
            kT_sb = ppool.tile([P, T // 2], fp16, name="kT")  # rows 64+: dup
            qT_sb = ppool.tile([P, T], fp16, name="qT")  # rows 64+: dup
            vT_tmp = ppool.tile([P, T // 2], fp16, name="vTt")  # rows 64+ used
            V_sb = ppool.tile([P, 16, E + 1], fp16, name="V")
            out_sb = ppool.tile([E + 1, T], f32, name="outsb")

            # PE warm-up: junk matmuls on a memset tile issue immediately
            # and hold the HAM clock-gate open until the first stripe lands.
            junk_in = ppool.tile([P, 512], fp16, name="junkin")
            nc.vector.memset(junk_in[:], 0.0)
            warm = ops.tile([E + 1, 512], f32, tag="po")
            for _ in range(NJUNK):
                nc.tensor.matmul(
                    warm[0:64, :],
                    junk_in[:, 0:E],
                    junk_in[:],
                    start=True,
                    stop=True,
                )

            # --- DMA triggers. sync ring (in transfer-need order):
            # wkv, xs0a, xs0b, xs1, [vt0, xs2, vt1, xs3, vt2, vt3, out7]
            wkv_sb = cpool.tile([P, DC, 2 * E], fp16)
            nc.sync.dma_start(wkv_sb[:], wkv.ap())
            xs0a_t = xt0pool.tile([P, 2, DC, HB], fp16, name="xs0t")
            nc.sync.dma_start(xs0a_t[:], xs0a.ap())
            xs0b_t = xt0pool.tile([P, 2, DC, HB], fp16, name="xs0t")
            nc.sync.dma_start(xs0b_t[:], xs0b.ap())
            xs_t = [None] * 4
            xs_t[1] = xtpool.tile([P, 4, DC, HB], fp16, name="xst")
            nc.sync.dma_start(xs_t[1][:], xs1.ap())
            # scalar ring: only wq2 (ACT queue stays clean for exp)
            wq2_sb = cpool.tile([P, DC, P], fp16)
            nc.scalar.dma_start(wq2_sb[:], wq2.ap())
            # gpsimd ring: small constants, then kT dups + early out stores
            maskD_sb = cpool.tile([P, 896], fp16)
            nc.gpsimd.dma_start(maskD_sb[:], maskD.ap())
            nc.gpsimd.dma_start(V_sb[:, :, E], ones.ap())  # ones column

            def kv_block_q0(m, src):
                # stripe-0 KV block m from quarter tensor src (own parity)
                kvh = kvps.tile([P, HB], f32, tag="proj")
                for dc in range(DC):
                    nc.tensor.matmul(
                        kvh[:],
                        wkv_sb[:, dc, :],
                        src[:, 0, dc, :],
                        start=(dc == 0),
                        stop=(dc == DC - 1),
                    )
                nc.vector.tensor_copy(kT_sb[0:E, _sl(HB * m, HB)], kvh[0:E, :])
                nc.vector.tensor_copy(vT_tmp[E:P, _sl(HB * m, HB)], kvh[E:P, :])
                # per-block K^T dup so slot masks aren't gated on the stripe
                nc.gpsimd.dma_start(
                    kT_sb[E:P, _sl(HB * m, HB)], kT_sb[0:E, _sl(HB * m, HB)]
                )

            def q_half_q0(half, src):
                q = kvps.tile([P, 512], f32, tag="proj")
                for dc in range(DC):
                    nc.tensor.matmul(
                        q[:],
                        wq2_sb[:, dc, :],
                        src[:, :, dc, :],
                        start=(dc == 0),
                        stop=(dc == DC - 1),
                    )
                nc.vector.tensor_copy(qT_sb[:, _sl(512 * half, 512)], q[:])

            def issue_stripe_proj(t2):
                # stripes 1..3: K^T|V^T one MM per d-chunk over both parity
                # blocks (strided rhs), then Q^T halves, then dup + V-transpose
                xts = xs_t[t2]
                xts_v = xts[:, :, :, :].rearrange("p (h par) c o -> p h par c o", h=2)
                kv = kvps.tile([P, 512], f32, tag="proj")
                for dc in range(DC):
                    nc.tensor.matmul(
                        kv[:],
                        wkv_sb[:, dc, :],
                        xts_v[:, :, 0, dc, :],
                        start=(dc == 0),
                        stop=(dc == DC - 1),
                    )
                m = 2 * t2
                nc.vector.tensor_copy(kT_sb[0:E, _sl(HB * m, 512)], kv[0:E, :])
                nc.vector.tensor_copy(vT_tmp[E:P, _sl(HB * m, 512)], kv[E:P, :])
                nc.gpsimd.dma_start(
                    kT_sb[E:P, _sl(512 * t2, 512)], kT_sb[0:E, _sl(512 * t2, 512)]
                )
                for half in range(2):
                    q = kvps.tile([P, 512], f32, tag="proj")
                    for dc in range(DC):
                        nc.tensor.matmul(
                            q[:],
                            wq2_sb[:, dc, :],
                            xts[:, _sl(2 * half, 2), dc, :],
                            start=(dc == 0),
                            stop=(dc == DC - 1),
                        )
                    qt_i = 2 * t2 + half
                    nc.vector.tensor_copy(qT_sb[:, _sl(512 * qt_i, 512)], q[:])
                issue_vt(t2)

            def issue_vt(t2):
                # V^T -> V via DMA xbar transpose (sync ring) into a
                # contiguous scratch (strided transpose dst corrupts data),
                # then one DVE copy into V_sb:
                # V_sb[p, 4*t2+j, e] = vT_tmp[64+e, 512*t2 + 128*j + p]
                scr = vspool.tile([P, 4, E], fp16, name="vscr")
                nc.sync.dma_start_transpose(
                    scr[:], vT_tmp[E:P, _sl(512 * t2, 512)]
                )
                nc.vector.tensor_copy(V_sb[:, _sl(4 * t2, 4), 0:E], scr[:])

            pendings = []  # (qt, oi, j0, nkb, exp_tile, po, w2)

            def issue_attnv(pend):
                qt, oi, j0, nkb, ex, po_t, w2 = pend
                n_groups = nkb // 2
                nc.tensor.matmul(
                    po_t[:],
                    V_sb[:, j0, :],
                    ex[:, 0:512],
                    start=(oi == 0),
                    stop=False,
                )
                nc.tensor.matmul(
                    po_t[:, 512 - w2 : 512],
                    V_sb[:, j0 + 1, :],
                    ex[:, _sl(512, w2)],
                    start=False,
                    stop=(oi == n_groups - 1),
                )
                if oi == n_groups - 1:
                    nc.vector.tensor_copy(out_sb[:, _sl(512 * qt, 512)], po_t[:])
                    eng = nc.sync if qt == NQT - 1 else nc.gpsimd
                    eng.dma_start(
                        out.ap()[:, _sl(512 * qt, 512)],
                        out_sb[:, _sl(512 * qt, 512)],
                    )

            def issue_slot(qt):
                nkb = 2 * qt + 2
                po = ops.tile([E + 1, 512], f32)
                # diagonal (masked) pair first: its mask latency hides
                # behind the remaining unmasked pairs. Its second k-tile
                # is trimmed to the causally-needed 384 q-columns.
                order = [nkb - 2] + list(range(0, nkb - 2, 2))
                for oi, j0 in enumerate(order):
                    w2 = 384 if oi == 0 else 512
                    ps = sps.tile([P, 1024], f32, tag="ps")
                    nc.tensor.matmul(
                        ps[:, 0:512],
                        kT_sb[0:E, _sl(P * j0, P)],
                        qT_sb[0:E, _sl(512 * qt, 512)],
                        start=True,
                        stop=True,
                        tile_position=(0, 0),
                    )
                    nc.tensor.matmul(
                        ps[:, _sl(512, w2)],
                        kT_sb[E:P, _sl(P * (j0 + 1), P)],
                        qT_sb[E:P, _sl(512 * qt + (512 - w2), w2)],
                        start=True,
                        stop=True,
                        tile_position=(64, 0),
                    )
                    ex = exppool.tile([P, 1024], fp16)
                    nc.scalar.activation(
                        ex[:, : 512 + w2], ps[:, : 512 + w2], Exp, scale=0.125
                    )
                    if oi == 0:
                        # single DVE multiply by the precomputed 0/1 mask
                        nc.vector.tensor_tensor(
                            out=ex[:, 0:896],
                            in0=ex[:, 0:896],
                            in1=maskD_sb[:],
                            op=Alu.mult,
                        )
                    pendings.append((qt, oi, j0, nkb, ex, po, w2))
                    if len(pendings) > 1:
                        issue_attnv(pendings.pop(0))

            # --- schedule
            # seg 0: stripe-0 projections (quarter-granular), slots 0,1
            kv_block_q0(0, xs0a_t)
            q_half_q0(0, xs0a_t)
            kv_block_q0(1, xs0b_t)
            q_half_q0(1, xs0b_t)
            issue_vt(0)
            issue_slot(0)
            issue_slot(1)
            # segs 1..3
            for seg in range(1, 4):
                if seg < 3:
                    xs_t[seg + 1] = xtpool.tile(
                        [P, 4, DC, HB], fp16, name="xst"
                    )
                    nc.sync.dma_start(
                        xs_t[seg + 1][:], (xs2 if seg == 1 else xs3).ap()
                    )
                issue_stripe_proj(seg)
                issue_slot(2 * seg)
                issue_slot(2 * seg + 1)
            for pend in pendings:
                issue_attnv(pend)

    nc.compile()
    return nc


def _build_maskD(h):
    """Diagonal-pair causal mask [P, 896] fp16 (1=keep).

    cols 0..511  : q-col c vs k-tile j0   -> iql[c]      >= 256h + p
    cols 512..895: q-col 128+i vs k-tile j0+1 -> iql[128+i] >= 256h + 128 + p
    where iql[c] = c (h=0) or c^256 (h=1, parity block swap).
    """
    iql = np.arange(512, dtype=np.int64)
    if h == 1:
        iql = iql ^ 256
    p = np.arange(P, dtype=np.int64)
    m = np.zeros((P, 896), dtype=np.float16)
    m[:, 0:512] = iql[None, :] >= (HB * h + p)[:, None]
    m[:, 512:896] = iql[None, 128:512] >= (HB * h + P + p)[:, None]
    return m


def _ensure_axon_hooks_stub():
    """bass_utils imports antenv.axon_hooks when BASS_TRACE is set; that
    module is absent in this image, so provide a no-op registry."""
    try:
        import antenv.axon_hooks  # noqa: F401
    except ImportError:
        m = types.ModuleType("antenv.axon_hooks")
        m._h = [None]
        m.set_axon_ntff_profile_hook = lambda h: m._h.__setitem__(0, h)
        m.get_axon_ntff_profile_hook = lambda: m._h[0]
        sys.modules["antenv.axon_hooks"] = m


def kernel(x, Wq, Wk, Wv):
    _ensure_axon_hooks_stub()
    from concourse.bass_utils import run_bass_kernel_spmd

    if "nc" not in _cache:
        _cache["nc"] = _build_program()
    nc = _cache["nc"]

    x = np.asarray(x, dtype=np.float32)
    Wq = np.asarray(Wq, dtype=np.float32)
    Wk = np.asarray(Wk, dtype=np.float32)
    Wv = np.asarray(Wv, dtype=np.float32)

    wkv = np.ascontiguousarray(
        np.concatenate([Wk, Wv], axis=1)
        .reshape(DC, P, 2 * E)
        .transpose(1, 0, 2)
        .astype(np.float16)
    )
    wq2 = np.ascontiguousarray(
        np.concatenate([Wq, Wq], axis=1)
        .reshape(DC, P, P)
        .transpose(1, 0, 2)
        .astype(np.float16)
    )
    ones = np.ones((P, 16), dtype=np.float16)
    maskDs = [_build_maskD(0), _build_maskD(1)]

    xT_all = x.transpose(0, 2, 1).astype(np.float16)  # [B, D, T]
    in_maps = []
    for c in range(NCORES):
        b, h = c // 2, c % 2
        xT = xT_all[b]
        if h == 1:  # swap 256-pairs so own-parity block is at even positions
            xT = xT.reshape(D, 8, 2, HB)[:, :, ::-1, :].reshape(D, T)
        # [dc, p, quarter, o] -> per-stripe [P, quarters, DC, HB] contiguous
        xq = xT.reshape(DC, P, 16, HB)
        im = {
            "xs0a": np.ascontiguousarray(xq[:, :, 0:2, :].transpose(1, 2, 0, 3)),
            "xs0b": np.ascontiguousarray(xq[:, :, 2:4, :].transpose(1, 2, 0, 3)),
            "wkv": wkv,
            "wq2": wq2,
            "maskD": maskDs[h],
            "ones": ones,
        }
        for s in range(1, 4):
            im[f"xs{s}"] = np.ascontiguousarray(
                xq[:, :, 4 * s : 4 * s + 4, :].transpose(1, 2, 0, 3)
            )
        in_maps.append(im)

    res = run_bass_kernel_spmd(nc, in_maps, list(range(NCORES)))
    _cache["last_res"] = res

    outp = np.empty((B, T, E), dtype=np.float32)
    for b in range(B):
        U = np.zeros((E + 1, T), dtype=np.float64)
        for h in range(2):
            u = res.results[2 * b + h]["out"].astype(np.float64)
            if h == 1:
                u = u.reshape(E + 1, 8, 2, HB)[:, :, ::-1, :].reshape(E + 1, T)
            U += u
        outp[b] = (U[:E] / U[E : E + 1]).T.astype(np.float32)
    return outp
